# revision 1
# baseline (speedup 1.0000x reference)
"""GatedCrossAttention kernel for 8 Trainium2 NeuronCores.

Sharding: the query/time dimension T (=2048) is split into 8 shards of 256;
each core runs the full fused gated-cross-attention for its T-shard across
all batches (key/weights replicated — k/v projections are cheap relative to
the T-dependent work, and this avoids any collective).
"""

import numpy as np
import jax
import jax.numpy as jnp

EMBED_DIM = 1024
ZDIM = 128
N_CORES = 8


def _compute(query, key, Wq, bq, Wk, bk, Wv, bv, Wh, bh, gamma, beta):
    E, Z = EMBED_DIM, ZDIM
    scaling = Z ** (-0.5)
    base = jnp.einsum('tbe,fe->tbf', query, Wq) + bq
    u = jax.nn.sigmoid(base[..., :E])
    rq = jax.nn.silu(base[..., E:])
    r = rq[..., :E]
    q = rq[..., E:] * gamma[0] + beta[0]
    k = jax.nn.silu(jnp.einsum('sbe,ze->sbz', key, Wk) + bk) * gamma[1] + beta[1]
    v = jax.nn.silu(jnp.einsum('sbe,fe->sbf', key, Wv) + bv)
    qk = jnp.einsum('tbz,sbz->bts', q * scaling, k)
    attn = jax.nn.softmax(qk, axis=-1)
    h = jnp.einsum('bts,sbf->tbf', attn, v)
    h = jnp.tanh(jnp.einsum('tbe,fe->tbf', h * r, Wh) + bh)
    return query + u * (h - query)


_pmapped = jax.pmap(
    _compute,
    in_axes=(0,) + (None,) * 11,
)


def kernel(**inputs) -> np.ndarray:
    query = np.asarray(inputs["query"], np.float32)
    T = query.shape[0]
    q_sh = query.reshape(N_CORES, T // N_CORES, *query.shape[1:])
    out = _pmapped(
        q_sh,
        jnp.asarray(inputs["key"], jnp.float32),
        jnp.asarray(inputs["Wq"], jnp.float32),
        jnp.asarray(inputs["bq"], jnp.float32),
        jnp.asarray(inputs["Wk"], jnp.float32),
        jnp.asarray(inputs["bk"], jnp.float32),
        jnp.asarray(inputs["Wv"], jnp.float32),
        jnp.asarray(inputs["bv"], jnp.float32),
        jnp.asarray(inputs["Wh"], jnp.float32),
        jnp.asarray(inputs["bh"], jnp.float32),
        jnp.asarray(inputs["gamma"], jnp.float32),
        jnp.asarray(inputs["beta"], jnp.float32),
    )
    return np.asarray(out).reshape(T, *query.shape[1:]).astype(np.float32)

